# revision 11
# baseline (speedup 1.0000x reference)
"""GCNII (8-layer GCN2Conv + BN) Trainium2 Bass kernel, 8-way node-sharded.

Strategy (per core, nodes partitioned into 8 ranges of N/8):
  - Edges assigned to the core owning their destination, sorted by
    128-dest block, padded per block to a uniform tile count.
  - H (node features, bf16) lives replicated in DRAM `HFULL` with
    per-core row ranges padded to a multiple of 128; `dma_gather`
    (MoE-style SWDGE gather) pulls the source row of each edge.
  - Segment-sum over edges is a one-hot matmul: for each 128-edge tile,
    PSUM[dest, feat] += S_ew[edge, dest]^T @ Hg[edge, feat], where
    S_ew carries the GCN edge weights (host-precomputed, bf16).
  - GCN2Conv identity mapping folded on host: W'_l = (1-beta_l) I + beta_l W_l,
    so out = p_blend @ W'_l is a single dense matmul (computed transposed
    so BN stats reduce along the free axis).
  - BN statistics cross-core via a [2,256] AllReduce; H exchange for the
    next layer via AllGather of the local bf16 rows.
"""
import os
import numpy as np
import ml_dtypes

import concourse.bacc as bacc
import concourse.mybir as mybir
from concourse.tile import TileContext
from concourse.bass_utils import run_bass_kernel_spmd

# ---------------- problem constants (hardcoded per task spec) ----------------
N = 10000
E = 320000
NFEAT = 512
NHID = 256
NCLASS = 64
NL = 8
ALPHA = 0.1
THETA = 0.5
BN_EPS = 1e-5

NCORES = 8
NPC = N // NCORES            # 1250 nodes per core
NB = (NPC + 127) // 128      # 10 dest blocks per core
NPAD = NB * 128              # 1280 padded local rows
NFULL = NCORES * NPAD        # 10240 padded rows in HFULL


def _chunks():
    """<=512-wide free-dim chunks covering NPAD."""
    out = []
    c0 = 0
    while c0 < NPAD:
        out.append((c0, min(512, NPAD - c0)))
        c0 += 512
    return out

FP = mybir.dt.float32
BF = mybir.dt.bfloat16
I16 = mybir.dt.int16

LAST_EXEC_NS = None
_PROGRAM_CACHE = {}


# ---------------------------- host preprocessing ----------------------------

def _preprocess(x, edge_index, lin0_w, lin0_b, lin1_w, lin1_b, conv_w,
                bn_gamma, bn_beta):
    loops = np.arange(N, dtype=edge_index.dtype)
    row = np.concatenate([edge_index[0], loops]).astype(np.int64)
    col = np.concatenate([edge_index[1], loops]).astype(np.int64)
    deg = np.bincount(col, minlength=N).astype(np.float64)
    dinv = np.where(deg > 0, 1.0 / np.sqrt(np.maximum(deg, 1e-12)), 0.0)
    ew = (dinv[row] * dinv[col]).astype(np.float32)

    # padded global row id inside HFULL for gather sources
    src_pad = row + (row // NPC) * (NPAD - NPC)

    per_core = []
    maxtiles = 0
    for c in range(NCORES):
        m = (col >= c * NPC) & (col < (c + 1) * NPC)
        esrc = src_pad[m]
        edst = col[m] - c * NPC
        eew = ew[m]
        blk = edst // 128
        order = np.argsort(blk, kind="stable")
        esrc, edst, eew, blk = esrc[order], edst[order], eew[order], blk[order]
        counts = np.bincount(blk, minlength=NB)
        maxtiles = max(maxtiles, int(np.ceil(counts.max() / 128.0)))
        per_core.append((esrc, edst, eew, blk, counts))

    T = maxtiles                 # tiles per dest block (uniform)
    NT = NB * T                  # tiles per core per layer
    NIDX = NT * 128
    # gather chunks: <=8 tiles (1024 idxs) per dma_gather call
    chunk_tiles = []
    t0 = 0
    while t0 < T:
        chunk_tiles.append(min(8, T - t0))
        t0 += min(8, T - t0)

    idx_packed_all = []
    s_all = []
    for c in range(NCORES):
        esrc, edst, eew, blk, counts = per_core[c]
        src_arr = np.zeros(NIDX, np.int64)
        ew_arr = np.zeros(NIDX, np.float32)
        dl_arr = np.zeros(NIDX, np.int64)
        off = 0
        for b in range(NB):
            n_b = int(counts[b])
            base = b * T * 128
            src_arr[base:base + n_b] = esrc[off:off + n_b]
            ew_arr[base:base + n_b] = eew[off:off + n_b]
            dl_arr[base:base + n_b] = edst[off:off + n_b] - b * 128
            off += n_b
        # S_ew [128, NT, 128] bf16
        j = np.arange(NIDX)
        s = np.zeros((128, NT, 128), ml_dtypes.bfloat16)
        s[j % 128, j // 128, dl_arr] = ew_arr.astype(ml_dtypes.bfloat16)
        # idx blocks: 16-row wrap per gather chunk, replicated to 128 rows
        block16 = np.zeros((16, NIDX // 16), np.int16)
        col0 = 0
        for b in range(NB):
            tt = 0
            for nt in chunk_tiles:
                n_chunk = nt * 128
                jj = np.arange(n_chunk)
                cj = (b * T + tt) * 128 + jj
                block16[jj % 16, col0 + jj // 16] = src_arr[cj].astype(np.int16)
                col0 += n_chunk // 16
                tt += nt
        idx_packed = np.tile(block16, (8, 1))
        idx_packed_all.append(idx_packed)
        s_all.append(s)

    # weights
    betas = np.log(THETA / np.arange(1, NL + 1) + 1.0).astype(np.float64)
    wp = np.zeros((128, NL, 2, NHID), ml_dtypes.bfloat16)
    for l in range(NL):
        wf = (betas[l] * conv_w[l].astype(np.float64)).astype(np.float32)
        wf = wf.reshape(2, 128, NHID)
        wp[:, l, :, :] = np.transpose(wf, (1, 0, 2)).astype(ml_dtypes.bfloat16)

    w0 = np.transpose(lin0_w.reshape(4, 128, NHID), (1, 0, 2)).astype(ml_dtypes.bfloat16)
    w1 = np.transpose(lin1_w.reshape(2, 128, NCLASS), (1, 0, 2)).astype(ml_dtypes.bfloat16)
    b0 = lin0_b.reshape(2, 128).T.astype(np.float32).copy()        # [128, 2]
    b1 = np.zeros((128, 1), np.float32)
    b1[:NCLASS, 0] = lin1_b.astype(np.float32)
    bng = np.transpose(bn_gamma.reshape(NL, 2, 128), (2, 0, 1)).astype(np.float32).copy()
    bnb = np.transpose(bn_beta.reshape(NL, 2, 128), (2, 0, 1)).astype(np.float32).copy()
    ident = np.eye(128, dtype=ml_dtypes.bfloat16)
    identf = np.eye(128, dtype=np.float32)

    # x^T per core, padded to NPAD, [128, 4, NPAD] bf16
    xt_all = []
    for c in range(NCORES):
        xl = x[c * NPC:(c + 1) * NPC].astype(np.float32)           # [1250, 512]
        xp = np.zeros((NPAD, NFEAT), np.float32)
        xp[:NPC] = xl
        xt = np.transpose(xp.reshape(NPAD, 4, 128), (2, 1, 0))     # [128, 4, NPAD]
        xt_all.append(np.ascontiguousarray(xt).astype(ml_dtypes.bfloat16))

    shared = dict(W0=w0, WP=wp, W1=w1, B0=b0, B1=b1, BNG=bng, BNB=bnb,
                  IDENT=ident, IDENTF=identf)
    in_maps = []
    for c in range(NCORES):
        m = dict(shared)
        m["XT"] = xt_all[c]
        m["SRC_IDX"] = idx_packed_all[c]
        m["S_IN"] = s_all[c]
        in_maps.append(m)
    return in_maps, T


# ----------------------------- device program -------------------------------

def _build_program(T):
    NT = NB * T
    NIDX = NT * 128
    CHUNKS = _chunks()
    chunk_tiles = []
    t0 = 0
    while t0 < T:
        chunk_tiles.append(min(8, T - t0))
        t0 += min(8, T - t0)
    betas = np.log(THETA / np.arange(1, NL + 1) + 1.0)

    nc = bacc.Bacc("TRN2", debug=False, num_devices=NCORES)

    XT = nc.dram_tensor("XT", [128, 4, NPAD], BF, kind="ExternalInput")
    SRC_IDX = nc.dram_tensor("SRC_IDX", [128, NIDX // 16], I16, kind="ExternalInput")
    S_IN = nc.dram_tensor("S_IN", [128, NT, 128], BF, kind="ExternalInput")
    W0 = nc.dram_tensor("W0", [128, 4, NHID], BF, kind="ExternalInput")
    WP = nc.dram_tensor("WP", [128, NL, 2, NHID], BF, kind="ExternalInput")
    W1 = nc.dram_tensor("W1", [128, 2, NCLASS], BF, kind="ExternalInput")
    B0 = nc.dram_tensor("B0", [128, 2], FP, kind="ExternalInput")
    B1 = nc.dram_tensor("B1", [128, 1], FP, kind="ExternalInput")
    BNG = nc.dram_tensor("BNG", [128, NL, 2], FP, kind="ExternalInput")
    BNB = nc.dram_tensor("BNB", [128, NL, 2], FP, kind="ExternalInput")
    IDENT = nc.dram_tensor("IDENT", [128, 128], BF, kind="ExternalInput")
    IDENTF = nc.dram_tensor("IDENTF", [128, 128], FP, kind="ExternalInput")
    OUT = nc.dram_tensor("OUT", [NCLASS, NPAD], FP, kind="ExternalOutput")

    AGIN = nc.dram_tensor("AGIN", [NPAD, NHID], BF)
    HFULL = nc.dram_tensor("HFULL", [NFULL, NHID], BF, addr_space="Shared")
    ARIN = nc.dram_tensor("ARIN", [128, 4], FP)
    AROUT = nc.dram_tensor("AROUT", [128, 4], FP, addr_space="Shared")

    rg = [list(range(NCORES))]
    cc_sem = nc.alloc_semaphore("cc_sem")
    cc_count = [0]

    with TileContext(nc) as tc:
        with (
            tc.tile_pool(name="const", bufs=1) as cpool,
            tc.tile_pool(name="hg", bufs=3) as hg_pool,
            tc.tile_pool(name="pbl", bufs=2) as pbl_pool,
            tc.tile_pool(name="big", bufs=1) as big_pool,
            tc.tile_pool(name="hT", bufs=2) as hT_pool,
            tc.tile_pool(name="stats", bufs=2) as st_pool,
            tc.tile_pool(name="psum_p", bufs=2, space="PSUM") as psumP,
            tc.tile_pool(name="psum_t", bufs=1, space="PSUM") as psumT,
            tc.tile_pool(name="psum_o", bufs=2, space="PSUM") as psumO,
        ):
            # ---- persistent constants
            s_sb = cpool.tile([128, NT, 128], BF)
            nc.sync.dma_start(out=s_sb[:], in_=S_IN[:])
            idx_sb = cpool.tile([128, NIDX // 16], I16)
            nc.sync.dma_start(out=idx_sb[:], in_=SRC_IDX[:])
            w0_sb = cpool.tile([128, 4, NHID], BF)
            nc.sync.dma_start(out=w0_sb[:], in_=W0[:])
            wp_sb = cpool.tile([128, NL, 2, NHID], BF)
            nc.sync.dma_start(out=wp_sb[:], in_=WP[:])
            w1_sb = cpool.tile([128, 2, NCLASS], BF)
            nc.sync.dma_start(out=w1_sb[:], in_=W1[:])
            b0_sb = cpool.tile([128, 2], FP)
            nc.sync.dma_start(out=b0_sb[:], in_=B0[:])
            b1_sb = cpool.tile([128, 1], FP)
            nc.sync.dma_start(out=b1_sb[:], in_=B1[:])
            bng_sb = cpool.tile([128, NL, 2], FP)
            nc.sync.dma_start(out=bng_sb[:], in_=BNG[:])
            bnb_sb = cpool.tile([128, NL, 2], FP)
            nc.sync.dma_start(out=bnb_sb[:], in_=BNB[:])
            ident_sb = cpool.tile([128, 128], BF)
            nc.sync.dma_start(out=ident_sb[:], in_=IDENT[:])
            identf_sb = cpool.tile([128, 128], FP)
            nc.sync.dma_start(out=identf_sb[:], in_=IDENTF[:])
            x0s_sb = cpool.tile([128, NB, NHID], FP)   # 0.1 * x0, row-major blocks

            agin_view = AGIN.ap().rearrange("(nb p) f -> p nb f", p=128)

            def collective(kind, op, in_ap, out_ap):
                with tc.tile_critical():
                    nc.gpsimd.collective_compute(
                        kind, op, replica_groups=rg, ins=[in_ap], outs=[out_ap],
                    ).then_inc(cc_sem)
                    cc_count[0] += 1
                    nc.gpsimd.wait_ge(cc_sem, cc_count[0])

            def emit_h(hT, with_x0s):
                """transpose hT [128,2,NPAD] -> H_loc rows, DMA to AGIN, AllGather."""
                h_loc = big_pool.tile([128, NB, NHID], BF, tag="h_loc")
                for nb in range(NB):
                    for g in range(2):
                        pT = psumT.tile([128, 128], BF)
                        nc.tensor.transpose(
                            pT[:], hT[:, g, nb * 128:(nb + 1) * 128], ident_sb[:])
                        nc.vector.tensor_copy(
                            h_loc[:, nb, g * 128:(g + 1) * 128], pT[:])
                        if with_x0s:
                            nc.scalar.mul(
                                x0s_sb[:, nb, g * 128:(g + 1) * 128], pT[:], ALPHA)
                nc.sync.dma_start(out=agin_view, in_=h_loc[:])
                collective("AllGather", mybir.AluOpType.bypass,
                           AGIN.ap().opt(), HFULL.ap().opt())

            # ---- lin0: h0^T = relu(W0^T x^T + b0)
            xt_sb = big_pool.tile([128, 4, NPAD], BF, tag="xt")
            nc.sync.dma_start(out=xt_sb[:], in_=XT[:])
            hT = hT_pool.tile([128, 2, NPAD], BF)
            for (c0, cw) in CHUNKS:
                po = psumO.tile([128, 2, 512], FP)
                for g in range(2):
                    for k in range(4):
                        nc.tensor.matmul(
                            po[:, g, 0:cw],
                            lhsT=w0_sb[:, k, g * 128:(g + 1) * 128],
                            rhs=xt_sb[:, k, c0:c0 + cw],
                            start=(k == 0), stop=(k == 3))
                    nc.scalar.activation(
                        hT[:, g, c0:c0 + cw], po[:, g, 0:cw],
                        mybir.ActivationFunctionType.Relu, bias=b0_sb[:, g:g + 1])
            emit_h(hT, with_x0s=True)

            # ---- conv layers
            for l in range(NL):
                pblT = big_pool.tile([128, 2, NPAD], BF, tag="pblT")
                pblTf = big_pool.tile([128, 2, NPAD], FP, tag="pblTf")
                for b in range(NB):
                    pp = psumP.tile([128, NHID], FP)
                    col0 = b * T * 128 // 16
                    tt = 0
                    for ci, ntiles in enumerate(chunk_tiles):
                        hg = hg_pool.tile([128, 8, NHID], BF)
                        nidx = ntiles * 128
                        nc.gpsimd.dma_gather(
                            out_ap=hg[:, 0:ntiles, :],
                            in_ap=HFULL.ap(),
                            idxs_ap=idx_sb[:, col0:col0 + nidx // 16],
                            num_idxs=nidx,
                            num_idxs_reg=nidx,
                            elem_size=NHID,
                        )
                        for t in range(ntiles):
                            nc.tensor.matmul(
                                pp[:],
                                lhsT=s_sb[:, b * T + tt + t, :],
                                rhs=hg[:, t, :],
                                start=(tt + t == 0),
                                stop=(tt + t == T - 1))
                        col0 += nidx // 16
                        tt += ntiles
                    # blend: pbl = (1-ALPHA)*pp + x0s_block  (x0s already *ALPHA)
                    pbl = pbl_pool.tile([128, NHID], FP)
                    nc.vector.scalar_tensor_tensor(
                        pbl[:], pp[:], 1.0 - ALPHA, x0s_sb[:, b, :],
                        op0=mybir.AluOpType.mult, op1=mybir.AluOpType.add)
                    for g in range(2):
                        pT = psumT.tile([128, 128], FP)
                        nc.tensor.transpose(
                            pT[:], pbl[:, g * 128:(g + 1) * 128], identf_sb[:])
                        nc.vector.tensor_copy(
                            pblTf[:, g, b * 128:(b + 1) * 128], pT[:])
                        nc.vector.tensor_copy(
                            pblT[:, g, b * 128:(b + 1) * 128], pT[:])
                # dense: out^T = (1-beta_l)*pblTf + (beta_l W)^T pblT
                outT = big_pool.tile([128, 2, NPAD], FP, tag="outT")
                for (c0, cw) in CHUNKS:
                    po = psumO.tile([128, 2, 512], FP)
                    for g in range(2):
                        for k in range(2):
                            nc.tensor.matmul(
                                po[:, g, 0:cw],
                                lhsT=wp_sb[:, l, k, g * 128:(g + 1) * 128],
                                rhs=pblT[:, k, c0:c0 + cw],
                                start=(k == 0), stop=(k == 1))
                        nc.vector.scalar_tensor_tensor(
                            outT[:, g, c0:c0 + cw],
                            pblTf[:, g, c0:c0 + cw], 1.0 - float(betas[l]),
                            po[:, g, 0:cw],
                            op0=mybir.AluOpType.mult, op1=mybir.AluOpType.add)
                # stats: cols [sum g0, sum g1, sumsq g0, sumsq g1]
                stat = st_pool.tile([128, 4], FP, tag="stat")
                sq = big_pool.tile([128, NPAD], FP, tag="sq")
                for g in range(2):
                    nc.vector.reduce_sum(
                        stat[:, g:g + 1], outT[:, g, :],
                        axis=mybir.AxisListType.X)
                    nc.scalar.activation(
                        sq[:], outT[:, g, :],
                        mybir.ActivationFunctionType.Square,
                        accum_out=stat[:, 2 + g:3 + g])
                nc.sync.dma_start(out=ARIN.ap(), in_=stat[:])
                collective("AllReduce", mybir.AluOpType.add,
                           ARIN.ap().opt(), AROUT.ap().opt())
                gstat = st_pool.tile([128, 4], FP, tag="gstat")
                nc.sync.dma_start(out=gstat[:], in_=AROUT.ap())
                # bn params: scale = g/sqrt(var+eps), shift = b - mean*scale
                mean = st_pool.tile([128, 2], FP, tag="mean")
                nc.scalar.mul(mean[:], gstat[:, 0:2], 1.0 / N)
                msq = st_pool.tile([128, 2], FP, tag="msq")
                nc.scalar.mul(msq[:], gstat[:, 2:4], 1.0 / N)
                m2 = st_pool.tile([128, 2], FP, tag="m2")
                nc.vector.tensor_mul(m2[:], mean[:], mean[:])
                var = st_pool.tile([128, 2], FP, tag="var")
                nc.vector.tensor_sub(var[:], msq[:], m2[:])
                veps = st_pool.tile([128, 2], FP, tag="veps")
                nc.vector.tensor_scalar_add(veps[:], var[:], BN_EPS)
                sd = st_pool.tile([128, 2], FP, tag="sd")
                nc.scalar.activation(
                    sd[:], veps[:], mybir.ActivationFunctionType.Sqrt)
                rsd = st_pool.tile([128, 2], FP, tag="rsd")
                nc.vector.reciprocal(rsd[:], sd[:])
                scale = st_pool.tile([128, 2], FP, tag="scale")
                nc.vector.tensor_mul(scale[:], rsd[:], bng_sb[:, l, :])
                ms = st_pool.tile([128, 2], FP, tag="ms")
                nc.vector.tensor_mul(ms[:], mean[:], scale[:])
                shift = st_pool.tile([128, 2], FP, tag="shift")
                nc.vector.tensor_sub(shift[:], bnb_sb[:, l, :], ms[:])
                # bn apply + relu
                hT = hT_pool.tile([128, 2, NPAD], BF)
                for g in range(2):
                    nc.scalar.activation(
                        hT[:, g, :], outT[:, g, :],
                        mybir.ActivationFunctionType.Relu,
                        scale=scale[:, g:g + 1], bias=shift[:, g:g + 1])
                if l < NL - 1:
                    emit_h(hT, with_x0s=False)

            # ---- lin1: out^T = W1^T h^T + b1
            o_sb = big_pool.tile([NCLASS, NPAD], FP, tag="o_sb")
            for (c0, cw) in CHUNKS:
                po = psumO.tile([128, 2, 512], FP)
                for k in range(2):
                    nc.tensor.matmul(
                        po[0:NCLASS, 0, 0:cw],
                        lhsT=w1_sb[:, k, :],
                        rhs=hT[:, k, c0:c0 + cw],
                        start=(k == 0), stop=(k == 1))
                nc.vector.tensor_scalar_add(
                    o_sb[:, c0:c0 + cw], po[0:NCLASS, 0, 0:cw],
                    b1_sb[0:NCLASS, :])
            nc.sync.dma_start(out=OUT[:], in_=o_sb[:])

    nc.compile()
    return nc


# --------------------------------- runner -----------------------------------

def kernel(x, edge_index, lin0_w, lin0_b, lin1_w, lin1_b, conv_w,
           bn_gamma, bn_beta):
    global LAST_EXEC_NS
    in_maps, T = _preprocess(
        np.asarray(x), np.asarray(edge_index), np.asarray(lin0_w),
        np.asarray(lin0_b), np.asarray(lin1_w), np.asarray(lin1_b),
        np.asarray(conv_w), np.asarray(bn_gamma), np.asarray(bn_beta))

    if T not in _PROGRAM_CACHE:
        _PROGRAM_CACHE[T] = _build_program(T)
    nc = _PROGRAM_CACHE[T]

    trace = bool(int(os.environ.get("KERNEL_TRACE", "0")))
    if trace:
        try:
            from ntff_shim import install
            install()
        except Exception:
            trace = False
    res = run_bass_kernel_spmd(nc, in_maps, list(range(NCORES)), trace=trace)
    LAST_EXEC_NS = res.exec_time_ns

    out = np.empty((N, NCLASS), np.float32)
    for c in range(NCORES):
        o = res.results[c]["OUT"]          # [NCLASS, NPAD]
        out[c * NPC:(c + 1) * NPC] = o[:, :NPC].T
    return out
